# revision 1
# baseline (speedup 1.0000x reference)
"""RNN-T Joint network kernel for Trainium2 (Bass/Tile), 8-core data-parallel.

Math (per batch b):
  hf = f[b] @ W1[:1024]            # (T=256, J=640)
  hg = g[b] @ W1[1024:]            # (U=65,  J=640)
  h[t,u,:]   = relu(hf[t] + hg[u] + b1)
  out[t,u,:] = h[t,u,:] @ W2 + b2  # (256, 65, 1024)

Sharding: data-parallel over B=8, one utterance per core.  Host-side prep
(part of the sharding step): all inputs are cast to bf16 and packed into
partition-major layouts so every input is a single large DMA with fully
contiguous >=2KB per-partition lines; the device program does no
transposes or dtype casts.

Device schedule (per core, u-major):
  - hfT[j, t] (f32) and hgT'[j, u] = hgT + b1 (f32) resident in SBUF
    (j on partitions), computed by PE from the pre-transposed inputs.
  - For each u: H_u[j, t] = relu(hfT[j, t] + hgT'[j, u]) built by ScalarE
    (bias = per-partition column hgT'[:, u]), cast to bf16.
  - PE: out_tile[t128, v512] += H_u[jc][:, t128].T @ W2[jc][:, v512],
    5 j-chunks accumulated in PSUM (fp32).
  - VectorE drains PSUM + adds broadcast b2 into a [128, 1024] tile,
    one DMA per (u, t-half) straight to HBM.
"""

import numpy as np

T, U = 256, 65
EH, PH, J, V = 1024, 320, 640, 1024
JC = J // 128           # 5 j-chunks
HC = EH // 128          # 8 h-chunks (f side)
GC = 3                  # g-side chunks (PH padded 320 -> 384 = 3*128)
N_CORES = 8

_CACHE = {}


def _build_nc():
    import concourse.bass as bass
    import concourse.bacc as bacc
    import concourse.mybir as mybir
    from concourse import tile

    f32 = mybir.dt.float32
    bf16 = mybir.dt.bfloat16
    Relu = mybir.ActivationFunctionType.Relu
    add = mybir.AluOpType.add

    nc = bacc.Bacc(None, target_bir_lowering=False)

    # packed, partition-major inputs (see _pack_* helpers); gside is
    # [gT | b1 | W1g] merged into one array -> one DMA
    GSIDE = GC * U + JC + JC * GC * 128
    fT_d = nc.declare_dram_parameter("fTp", [128, HC * T], bf16, isOutput=False)
    gs_d = nc.declare_dram_parameter("gside", [128, GSIDE], bf16, isOutput=False)
    W1f_d = nc.declare_dram_parameter("W1fp", [128, JC * HC * 128], bf16,
                                      isOutput=False)
    W2_d = nc.declare_dram_parameter("W2p", [128, JC * V], bf16, isOutput=False)
    b2_d = nc.declare_dram_parameter("b2p", [1, V], bf16, isOutput=False)
    out_d = nc.declare_dram_parameter("out", [T, U, V], f32, isOutput=True)

    with tile.TileContext(nc) as tc:
        with tc.tile_pool(name="const", bufs=1) as cpool:
            # Preload the ScalarE activation table (Relu) off the critical
            # path: the first act instruction pays ~1.3us table load.
            dumin = cpool.tile([128, 1], f32)
            nc.gpsimd.memset(dumin[:], 0.0)
            dumout = cpool.tile([128, 1], f32)
            nc.scalar.activation(dumout[:], dumin[:], Relu, bias=0.0, scale=1.0)

            # ---------------- DMA (priority order) ----------------
            # Few large DMAs via SP/HWDGE, ordered so each PE consumer's
            # input lands just before the (ramp-paced) PE stream reaches it.
            # fT/W1f[c0] split in halves so the first hfT matmuls start
            # ~1us earlier; b2 (tiny) feeds the b2-broadcast matmuls that
            # fill the gap until fT's second half lands.
            fTall = cpool.tile([128, HC * T], bf16)
            W1fall = cpool.tile([128, JC * HC * 128], bf16)
            gsall = cpool.tile([128, GSIDE], bf16)
            W2all = cpool.tile([128, JC * V], bf16)
            b2row = cpool.tile([1, V], bf16)

            half_f = 5 * T
            half_w = 5 * 128

            def dma_w1f(c):
                nc.sync.dma_start(
                    out=W1fall[:, c * HC * 128:(c + 1) * HC * 128],
                    in_=W1f_d[:, c * HC * 128:(c + 1) * HC * 128])

            def dma_w2(c):
                nc.sync.dma_start(out=W2all[:, c * V:(c + 1) * V],
                                  in_=W2_d[:, c * V:(c + 1) * V])

            nc.sync.dma_start(out=fTall[:, :half_f], in_=fT_d[:, :half_f])
            nc.sync.dma_start(out=W1fall[:, :half_w], in_=W1f_d[:, :half_w])
            # tiny b2 row via Pool/SWDGE: frees an HWDGE ladder slot and its
            # 11ns transfer jumping the DMA queue is harmless
            nc.gpsimd.dma_start(out=b2row[:], in_=b2_d[:])
            # W1f chunk1 before the fT/W1fc0 second halves: hfT1 is the
            # tighter consumer (b2bc + h0-4 cover the wait for h5-7)
            dma_w1f(1)
            nc.sync.dma_start(out=W1fall[:, half_w:HC * 128],
                              in_=W1f_d[:, half_w:HC * 128])
            nc.sync.dma_start(out=fTall[:, half_f:], in_=fT_d[:, half_f:])
            dma_w1f(2)
            gs_half = GC * U + JC + 3 * GC * 128
            nc.sync.dma_start(out=gsall[:, :gs_half], in_=gs_d[:, :gs_half])
            nc.sync.dma_start(out=gsall[:, gs_half:], in_=gs_d[:, gs_half:])
            dma_w1f(3)
            dma_w2(0)
            dma_w1f(4)
            dma_w2(1)
            dma_w2(2)
            dma_w2(3)
            dma_w2(4)

            def fT(h):
                return fTall[:, h * T:(h + 1) * T]

            def w1f(c, h):
                o = (c * HC + h) * 128
                return W1fall[:, o:o + 128]

            def gT(pc):
                return gsall[:, pc * U:(pc + 1) * U]

            b1bf = gsall[:, GC * U:GC * U + JC]
            b1sb = cpool.tile([128, JC], f32)

            def w1g(c, pc):
                o = GC * U + JC + (c * GC + pc) * 128
                return gsall[:, o:o + 128]

            def w2(c, lo, hi):
                return W2all[:, c * V + lo:c * V + hi]

            # ------------- single gapless PE stream -------------
            # One PSUM pool (4 tags x 2 bufs x [128,512]f32 = all 8 banks).
            # The first-layer chunks, the b2 broadcast, and u0's main-GEMM
            # groups are interleaved so the PE never idles between its first
            # matmul and the end of the kernel (the cost model halves PE
            # speed for ~3us after any idle gap, so gaps are doubly costly).
            # u0's tt1 groups are skewed one chunk behind tt0 so the tt0
            # banks drain in time for u1 to start seamlessly.
            hfTs = [None] * JC   # f32 [128, T]  (hf^T)
            hgTs = [None] * JC   # f32 [128, U]  (hg^T + b1)

            with (
                tc.tile_pool(name="hpool", bufs=4) as hpool,
                tc.tile_pool(name="opool", bufs=3) as opool,
                tc.tile_pool(name="mpsum", bufs=2, space=bass.MemorySpace.PSUM) as mpsum,
            ):
                def psum(tag):
                    return mpsum.tile([128, 512], f32, tag=tag, name=f"ps_{tag}")

                def hfT_mms(pf, c, h0, h1):
                    for h in range(h0, h1):
                        nc.tensor.matmul(pf[:, :T], w1f(c, h), fT(h),
                                         start=(h == 0), stop=(h == HC - 1))

                def hfT_drain(pf, c):
                    t = cpool.tile([128, T], f32, tag=f"hfT{c}")
                    nc.vector.tensor_copy(t[:], pf[:, :T])
                    hfTs[c] = t

                def hfT_chunk(c, tag):
                    pf = psum(tag)
                    hfT_mms(pf, c, 0, HC)
                    hfT_drain(pf, c)

                def hgT_chunk(c, tag):
                    # hg^T[c] = sum_pc W1g[c][pc].T @ gT[pc]  (+ b1 on drain)
                    ph = psum(tag)
                    for pc in range(GC):
                        nc.tensor.matmul(ph[:, :U], w1g(c, pc), gT(pc),
                                         start=(pc == 0), stop=(pc == GC - 1))
                    t = cpool.tile([128, U], f32, tag=f"hgT{c}")
                    nc.vector.tensor_scalar(t[:], ph[:, :U], b1sb[:, c:c + 1],
                                            None, add)
                    hgTs[c] = t

                def act(u, c):
                    ht = hpool.tile([128, T], bf16, tag=f"H{c}")
                    nc.scalar.activation(ht[:], hfTs[c][:], Relu,
                                         bias=hgTs[c][:, u:u + 1], scale=1.0)
                    return ht

                ones = cpool.tile([1, 128], bf16)
                nc.vector.memset(ones[:], 1.0)

                # PE warm-up: a tiny matmul gated only by the Pool-engine
                # memset (~0.25us) starts the PE busy-clock early, so the
                # real matmuls (first one ~3.8us in) run past the slow ramp
                # window at full rate.
                warm = psum("t1")
                nc.tensor.matmul(warm[0:1, 0:1], dumin[0:1, 0:1],
                                 dumin[0:1, 0:1], start=True, stop=True)
                wscr = cpool.tile([128, 1], f32)
                nc.vector.tensor_copy(wscr[0:1, :], warm[0:1, 0:1])
                b2bc = cpool.tile([128, V], f32)

                # hfT0 first half; b2 broadcast (needs only the tiny b2 row)
                # fills the gap until fT's second half lands.  All hgT
                # chunks right after (their merged input lands early) so the
                # u0 act chain on ScalarE has maximum lead time.
                pf0 = psum("t0")
                hfT_mms(pf0, 0, 0, 5)
                for vh, tag in ((0, "t2"), (1, "t3")):
                    pb = psum(tag)
                    nc.tensor.matmul(pb[:], ones[:],
                                     b2row[:, vh * 512:(vh + 1) * 512],
                                     start=True, stop=True)
                    nc.vector.tensor_copy(b2bc[:, vh * 512:(vh + 1) * 512], pb[:])
                pf1 = psum("t1")
                hfT_mms(pf1, 1, 0, 5)
                hfT_mms(pf0, 0, 5, HC)
                hfT_drain(pf0, 0)
                hfT_mms(pf1, 1, 5, HC)
                hfT_drain(pf1, 1)
                hfT_chunk(2, "t2")
                nc.vector.tensor_copy(b1sb[:], b1bf)
                hgT_chunk(0, "t3")
                hgT_chunk(1, "t0")
                hgT_chunk(2, "t1")
                hgT_chunk(3, "t2")
                hgT_chunk(4, "t3")
                H0 = [None] * JC
                H0[0] = act(0, 0)
                H0[1] = act(0, 1)
                H0[2] = act(0, 2)
                hfT_chunk(3, "t0")
                H0[3] = act(0, 3)

                def u0mm(g, c, tt, vh, start, stop):
                    nc.tensor.matmul(g[:], H0[c][:, tt * 128:(tt + 1) * 128],
                                     w2(c, vh * 512, (vh + 1) * 512),
                                     start=start, stop=stop)

                # u0 skewed stages: tt0 groups (g00,g01) run chunk c while
                # tt1 groups (g10,g11) run chunk c-1.
                g00 = psum("t1")
                g01 = psum("t2")
                u0mm(g00, 0, 0, 0, True, False)     # S0
                u0mm(g01, 0, 0, 1, True, False)
                hfT_chunk(4, "t3")
                H0[4] = act(0, 4)
                g10 = psum("t0")
                g11 = psum("t1")
                u0mm(g00, 1, 0, 0, False, False)    # S1
                u0mm(g01, 1, 0, 1, False, False)
                u0mm(g10, 0, 1, 0, True, False)
                u0mm(g11, 0, 1, 1, True, False)
                for c in (2, 3):                    # S2, S3
                    u0mm(g00, c, 0, 0, False, False)
                    u0mm(g01, c, 0, 1, False, False)
                    u0mm(g10, c - 1, 1, 0, False, False)
                    u0mm(g11, c - 1, 1, 1, False, False)
                u0mm(g00, 4, 0, 0, False, True)     # S4
                u0mm(g01, 4, 0, 1, False, True)
                u0mm(g10, 3, 1, 0, False, False)
                u0mm(g11, 3, 1, 1, False, False)
                ot0 = opool.tile([128, V], f32, tag="o0")
                nc.vector.tensor_tensor(ot0[:, 0:512], g00[:], b2bc[:, 0:512], add)
                nc.vector.tensor_tensor(ot0[:, 512:V], g01[:], b2bc[:, 512:V], add)
                nc.sync.dma_start(out=out_d[0:128, 0, :], in_=ot0[:])
                u0mm(g10, 4, 1, 0, False, True)     # S5
                u0mm(g11, 4, 1, 1, False, True)
                ot1 = opool.tile([128, V], f32, tag="o1")
                nc.vector.tensor_tensor(ot1[:, 0:512], g10[:], b2bc[:, 0:512], add)
                nc.vector.tensor_tensor(ot1[:, 512:V], g11[:], b2bc[:, 512:V], add)
                nc.sync.dma_start(out=out_d[128:256, 0, :], in_=ot1[:])

                # ---------------- u = 1 .. U-1 ----------------
                # Tag map keyed so each group's bank was last used either by
                # an early-drained prologue chunk or by a group two rounds
                # back (bufs=2 rotation).
                TAG = {(0, 0): "t2", (0, 1): "t3", (1, 0): "t1", (1, 1): "t0"}
                for u in range(1, U - 1):
                    Hs = [act(u, c) for c in range(JC)]
                    for tt in range(2):
                        ps = [psum(TAG[(tt, 0)]), psum(TAG[(tt, 1)])]
                        for c in range(JC):
                            lhsT = Hs[c][:, tt * 128:(tt + 1) * 128]
                            nc.tensor.matmul(ps[0][:], lhsT, w2(c, 0, 512),
                                             start=(c == 0), stop=(c == JC - 1))
                            nc.tensor.matmul(ps[1][:], lhsT, w2(c, 512, 1024),
                                             start=(c == 0), stop=(c == JC - 1))
                        ot = opool.tile([128, V], f32, tag=f"o{tt}")
                        for vh in range(2):
                            nc.vector.tensor_tensor(
                                ot[:, vh * 512:(vh + 1) * 512], ps[vh][:],
                                b2bc[:, vh * 512:(vh + 1) * 512], add)
                        nc.sync.dma_start(
                            out=out_d[tt * 128:(tt + 1) * 128, u, :],
                            in_=ot[:])

                # ---------------- final u: short tail ----------------
                u = U - 1
                Hs = [act(u, c) for c in range(JC)]
                for tt in range(2):
                    ps = [psum(TAG[(tt, 0)]), psum(TAG[(tt, 1)])]
                    for c in range(JC):
                        lhsT = Hs[c][:, tt * 128:(tt + 1) * 128]
                        nc.tensor.matmul(ps[0][:], lhsT, w2(c, 0, 512),
                                         start=(c == 0), stop=(c == JC - 1))
                        nc.tensor.matmul(ps[1][:], lhsT, w2(c, 512, 1024),
                                         start=(c == 0), stop=(c == JC - 1))
                    ot = opool.tile([128, V], f32, tag=f"o{tt}")
                    for vh in range(2):
                        nc.vector.tensor_tensor(
                            ot[:, vh * 512:(vh + 1) * 512], ps[vh][:],
                            b2bc[:, vh * 512:(vh + 1) * 512], add)
                        # split the final stores across two HWDGE queues so
                        # the post-last-matmul drain+DMA chain is shorter
                        eng = nc.sync if vh == 0 else nc.scalar
                        eng.dma_start(
                            out=out_d[tt * 128:(tt + 1) * 128, u,
                                      vh * 512:(vh + 1) * 512],
                            in_=ot[:, vh * 512:(vh + 1) * 512])
    nc.compile()
    return nc


def _get_nc():
    if "nc" not in _CACHE:
        _CACHE["nc"] = _build_nc()
    return _CACHE["nc"]


def _pack_shared(W1, b1, W2):
    """Partition-major packed weights, shared across cores (bf16)."""
    import ml_dtypes

    bf16 = ml_dtypes.bfloat16
    W1 = np.asarray(W1, dtype=np.float32)
    W2 = np.asarray(W2, dtype=np.float32)
    # W1f: [p, c, h, k] with source index [h*128+p, c*128+k]
    W1fp = np.ascontiguousarray(
        W1[:EH].reshape(HC, 128, JC, 128).transpose(1, 2, 0, 3)
        .reshape(128, JC * HC * 128)).astype(bf16)
    # W1g: pad rows to 384, then [p, c, pc, k]
    W1g = np.zeros((GC * 128, J), dtype=np.float32)
    W1g[:PH] = W1[EH:]
    W1gp = np.ascontiguousarray(
        W1g.reshape(GC, 128, JC, 128).transpose(1, 2, 0, 3)
        .reshape(128, JC * GC * 128)).astype(bf16)
    # W2: [p, c, v] with source [c*128+p, v]
    W2p = np.ascontiguousarray(
        W2.reshape(JC, 128, V).transpose(1, 0, 2).reshape(128, JC * V)
    ).astype(bf16)
    b1p = np.asarray(b1, dtype=np.float32).reshape(JC, 128).T.astype(bf16)
    return W1fp, W1gp, W2p, b1p


def _pack_core(f_b, g_b, b1p, W1gp):
    import ml_dtypes

    bf16 = ml_dtypes.bfloat16
    # fT packed: [p, h, t] with source f[t, h*128+p]
    fTp = np.ascontiguousarray(
        f_b.T.reshape(HC, 128, T).transpose(1, 0, 2).reshape(128, HC * T)
    ).astype(bf16)
    # gT packed: pad rows of g^T [PH, U] to 384 = GC*128
    gTfull = np.zeros((GC * 128, U), dtype=np.float32)
    gTfull[:PH] = g_b.T
    gTp = (gTfull.reshape(GC, 128, U).transpose(1, 0, 2)
           .reshape(128, GC * U).astype(bf16))
    # one merged g-side array: [gT | b1 | W1g]
    gside = np.ascontiguousarray(np.concatenate([gTp, b1p, W1gp], axis=1))
    return fTp, gside


def run(f, g, W1, b1, W2, b2, trace=False):
    """Returns (full_output, BassKernelResults)."""
    import ml_dtypes
    from concourse.bass_utils import run_bass_kernel_spmd

    nc = _get_nc()

    W1fp, W1gp, W2p, b1p = _pack_shared(W1, b1, W2)
    b2p = np.ascontiguousarray(
        np.asarray(b2, dtype=np.float32).reshape(1, V)).astype(ml_dtypes.bfloat16)
    f = np.asarray(f, dtype=np.float32)
    g = np.asarray(g, dtype=np.float32)

    in_maps = []
    for i in range(N_CORES):
        fTp, gside = _pack_core(f[i], g[i], b1p, W1gp)
        in_maps.append({
            "fTp": fTp,
            "gside": gside,
            "W1fp": W1fp,
            "W2p": W2p,
            "b2p": b2p,
        })
    res = run_bass_kernel_spmd(nc, in_maps, list(range(N_CORES)), trace=trace)
    out = np.stack([res.results[i]["out"] for i in range(N_CORES)], axis=0)
    return out, res


def kernel(f, g, W1, b1, W2, b2):
    out, _ = run(f, g, W1, b1, W2, b2)
    return out



# revision 40
# speedup vs baseline: 1.5954x; 1.5954x over previous
"""RNN-T Joint network kernel for Trainium2 (Bass/Tile), 8-core data-parallel.

Math (per batch b):
  hf = f[b] @ W1[:1024]            # (T=256, J=640)
  hg = g[b] @ W1[1024:]            # (U=65,  J=640)
  h[t,u,:]   = relu(hf[t] + hg[u] + b1)
  out[t,u,:] = h[t,u,:] @ W2 + b2  # (256, 65, 1024)

Sharding: data-parallel over B=8, one utterance per core.  Host-side prep
(part of the sharding step): inputs are cast/packed into partition-major
layouts; weights for the fp8 path are pre-split hi/lo; outputs come back
fp16 and are upcast on the host.

Device schedule (per core, u-major):
  - hfT[j, t] (f32) and hgT'[j, u] = hgT + b1 (f32, x0.25 for fp8 tiles)
    resident in SBUF (j on partitions), computed by PE (bf16 operands).
  - The layer-2 contraction (J=640 = 5 k-tiles of 128) is split: the first
    (5-K8) tiles stay bf16; the last K8 tiles use fp8e4m3 with DoubleRow
    perf mode (2 k-tiles per PE pass at 0.5 cycles/row).  To keep accuracy,
    H is quantized once per tile (relu act emits fp8 directly, scale 1/4)
    while W2 is pre-split into hi+lo fp8 pairs (scale 4) so its
    quantization error cancels to ~bf16 level.  Measured end-to-end max
    rel err ~1.6-1.9e-2 scale vs the 2e-2 gate (K8=3 / K8=4).
  - Per u: ScalarE builds H tiles (bf16) and the fp8 slot tile, PE runs
    2x2 (tt x vh) PSUM groups of [128,512], DVE drains psum + b2 into
    fp16 out tiles, one DMA per (u, tt) straight to HBM.
"""

import numpy as np

T, U = 256, 65
EH, PH, J, V = 1024, 320, 640, 1024
JC = J // 128           # 5 j-chunks
HC = EH // 128          # 8 h-chunks (f side)
GC = 3                  # g-side chunks (PH padded 320 -> 384 = 3*128)
N_CORES = 8

K8 = 4                  # number of fp8 k-tiles (from the top); 5-K8 stay bf16
NBF = JC - K8

_CACHE = {}


def _build_nc():
    import concourse.bass as bass
    import concourse.bacc as bacc
    import concourse.mybir as mybir
    from concourse import tile

    f32 = mybir.dt.float32
    f16 = mybir.dt.float16
    bf16 = mybir.dt.bfloat16
    e4 = mybir.dt.float8e4
    Relu = mybir.ActivationFunctionType.Relu
    add = mybir.AluOpType.add
    mult = mybir.AluOpType.mult
    DR = mybir.MatmulPerfMode.DoubleRow

    nc = bacc.Bacc(None, target_bir_lowering=False)

    # packed, partition-major inputs (see _pack_* helpers); gside is
    # [gT | b1 | W1g] merged into one array -> one DMA
    GSIDE = GC * U + JC + JC * GC * 128
    # fp8 W2 pair tiles: K8=4 -> (hi12, lo12, hi34, lo34); K8=3 ->
    # (hi23, lo23, hi4lo4)
    NPAIR = K8 if K8 == 4 else 3
    fT_d = nc.declare_dram_parameter("fTp", [128, HC * T], bf16, isOutput=False)
    gs_d = nc.declare_dram_parameter("gside", [128, GSIDE], bf16, isOutput=False)
    W1f_d = nc.declare_dram_parameter("W1fp", [128, JC * HC * 128], bf16,
                                      isOutput=False)
    W2b_d = nc.declare_dram_parameter("W2bf", [128, NBF * V], bf16,
                                      isOutput=False)
    W2q_d = nc.declare_dram_parameter("W2q", [128, NPAIR * 2 * V], e4,
                                      isOutput=False)
    b2_d = nc.declare_dram_parameter("b2p", [1, V], bf16, isOutput=False)
    out_d = nc.declare_dram_parameter("out", [T, U, V], f16, isOutput=True)

    with tile.TileContext(nc) as tc:
        with tc.tile_pool(name="const", bufs=1) as cpool:
            # Preload the ScalarE activation table (Relu) off the critical
            # path: the first act instruction pays ~1.3us table load.
            dumin = cpool.tile([128, 1], f32)
            nc.gpsimd.memset(dumin[:], 0.0)
            dumout = cpool.tile([128, 1], f32)
            nc.scalar.activation(dumout[:], dumin[:], Relu, bias=0.0, scale=1.0)

            # ---------------- DMA (priority order) ----------------
            # Few large DMAs via SP/HWDGE, ordered so each PE consumer's
            # input lands just before the (ramp-paced) PE stream reaches it.
            fTall = cpool.tile([128, HC * T], bf16)
            W1fall = cpool.tile([128, JC * HC * 128], bf16)
            gsall = cpool.tile([128, GSIDE], bf16)
            W2ball = (cpool.tile([128, NBF * V], bf16, name="W2ball")
                      if NBF else None)
            # one 3D tile per fp8 pair: [128, 2(k-tiles), V] — DoubleRow
            # needs the k-pair dim at AP position 1
            W2qp = [cpool.tile([128, 2, V], e4, name=f"W2qp{p}")
                    for p in range(NPAIR)]
            b2row = cpool.tile([1, V], bf16)
            b2bc = cpool.tile([128, V], f32)

            half_f = 5 * T
            half_w = 5 * 128

            def dma_w1f(c):
                nc.sync.dma_start(
                    out=W1fall[:, c * HC * 128:(c + 1) * HC * 128],
                    in_=W1f_d[:, c * HC * 128:(c + 1) * HC * 128])

            def dma_w2q(p):
                nc.sync.dma_start(
                    out=W2qp[p][:],
                    in_=W2q_d[:, p * 2 * V:(p + 1) * 2 * V])

            nc.sync.dma_start(out=fTall[:, :half_f], in_=fT_d[:, :half_f])
            nc.sync.dma_start(out=W1fall[:, :half_w], in_=W1f_d[:, :half_w])
            # tiny b2 row via Pool/SWDGE: off the HWDGE ladder, and its
            # 11ns transfer jumping the DMA queue is harmless; PE
            # broadcasts it across partitions during the prologue.
            nc.gpsimd.dma_start(out=b2row[:], in_=b2_d[:])
            dma_w1f(1)
            nc.sync.dma_start(out=W1fall[:, half_w:HC * 128],
                              in_=W1f_d[:, half_w:HC * 128])
            nc.sync.dma_start(out=fTall[:, half_f:], in_=fT_d[:, half_f:])
            dma_w1f(2)
            nc.sync.dma_start(out=gsall[:], in_=gs_d[:])
            for c in range(NBF):
                nc.sync.dma_start(out=W2ball[:, c * V:(c + 1) * V],
                                  in_=W2b_d[:, c * V:(c + 1) * V])
            dma_w1f(3)
            dma_w2q(0)
            dma_w2q(1)
            dma_w1f(4)
            for p in range(2, NPAIR):
                dma_w2q(p)

            def fT(h):
                return fTall[:, h * T:(h + 1) * T]

            def w1f(c, h):
                o = (c * HC + h) * 128
                return W1fall[:, o:o + 128]

            def gT(pc):
                return gsall[:, pc * U:(pc + 1) * U]

            b1bf = gsall[:, GC * U:GC * U + JC]
            b1sb = cpool.tile([128, JC], f32)

            def w1g(c, pc):
                o = GC * U + JC + (c * GC + pc) * 128
                return gsall[:, o:o + 128]

            def w2b(c, lo, hi):
                return W2ball[:, c * V + lo:c * V + hi]

            # ------------- single gapless PE stream -------------
            # One PSUM pool (2 tags x 2 bufs x [128,1024]f32 two-bank tiles
            # = all 8 banks).  Both vh groups of a tt live in one tile so
            # the drain is a single [128,1024] DVE op (1192ns vs 2x658):
            # with the fp8-shortened PE stream (2560ns/u) two 658ns drains
            # per tt would make DVE the bottleneck.
            hfTs = [None] * JC   # f32 [128, T]  (hf^T)
            hgTs = [None] * JC   # f32 [128, U]  (hg^T + b1; x0.25 on fp8 tiles)

            with (
                tc.tile_pool(name="hpool", bufs=4) as hpool,
                tc.tile_pool(name="opool", bufs=3) as opool,
                tc.tile_pool(name="mpsum", bufs=2, space=bass.MemorySpace.PSUM) as mpsum,
            ):
                def psum(tag):
                    return mpsum.tile([128, 1024], f32, tag=tag,
                                      name=f"ps_{tag}")

                def hfT_mms(pf, col0, c, h0, h1):
                    for h in range(h0, h1):
                        nc.tensor.matmul(pf[:, col0:col0 + T], w1f(c, h),
                                         fT(h),
                                         start=(h == 0), stop=(h == HC - 1))

                def hfT_drain(pf, col0, c):
                    t = cpool.tile([128, T], f32, tag=f"hfT{c}")
                    nc.vector.tensor_copy(t[:], pf[:, col0:col0 + T])
                    hfTs[c] = t

                def hfT_chunk(c, pf, col0):
                    hfT_mms(pf, col0, c, 0, HC)
                    hfT_drain(pf, col0, c)

                def hgT_chunk(c, ph, col0):
                    # hg^T[c] = sum_pc W1g[c][pc].T @ gT[pc]  (+ b1 on drain;
                    # fp8 tiles also fold the 1/4 act pre-scale in here)
                    for pc in range(GC):
                        nc.tensor.matmul(ph[:, col0:col0 + U], w1g(c, pc),
                                         gT(pc),
                                         start=(pc == 0), stop=(pc == GC - 1))
                    t = cpool.tile([128, U], f32, tag=f"hgT{c}")
                    if c >= NBF:
                        nc.vector.tensor_scalar(t[:], ph[:, col0:col0 + U],
                                                b1sb[:, c:c + 1], 0.25,
                                                add, mult)
                    else:
                        nc.vector.tensor_scalar(t[:], ph[:, col0:col0 + U],
                                                b1sb[:, c:c + 1], None, add)
                    hgTs[c] = t

                # H tiles for u: bf16 tiles (c < NBF) separate; fp8 tiles
                # in one [128, 4, T] slot tile (K8=3 duplicates tile 4 in
                # slots 2,3 so DoubleRow pairs stay free-dim-adjacent).
                SLOT_C = [1, 2, 3, 4] if K8 == 4 else [2, 3, 4, 4]

                def act_u(u, slots=range(4), hb_hq=None):
                    if hb_hq is None:
                        Hb = []
                        for c in range(NBF):
                            ht = hpool.tile([128, T], bf16, tag=f"H{c}")
                            nc.scalar.activation(ht[:], hfTs[c][:], Relu,
                                                 bias=hgTs[c][:, u:u + 1],
                                                 scale=1.0)
                            Hb.append(ht)
                        hq = hpool.tile([128, 4, T], e4, tag="HQ")
                    else:
                        Hb, hq = hb_hq
                    for si in slots:
                        c = SLOT_C[si]
                        nc.scalar.activation(hq[:, si, :], hfTs[c][:], Relu,
                                             bias=hgTs[c][:, u:u + 1],
                                             scale=0.25)
                    return Hb, hq

                # Matmul sequence per (tt, vh) psum group: NBF bf16 k-tiles
                # then the fp8 DoubleRow pairs.  W2qall pair p holds
                # [128, 2, V]; K8=4: p0=hi(1,2) p1=lo(1,2) p2=hi(3,4)
                # p3=lo(3,4); K8=3: p0=hi(2,3) p1=lo(2,3) p2=(hi4,lo4).
                if K8 == 4:
                    DR_SEQ = [(0, 0), (0, 1), (2, 2), (2, 3)]  # (slot0, pair)
                else:
                    DR_SEQ = [(0, 0), (0, 1), (2, 2)]
                NMM = NBF + len(DR_SEQ)

                def group_mms(specs, Hb, hq, tt, mi):
                    # emit matmul index mi for each (psumAP, vlo, vwid) group
                    ts = slice(tt * 128, (tt + 1) * 128)
                    for ps, lo, wid in specs:
                        if mi < NBF:
                            nc.tensor.matmul(
                                ps, Hb[mi][:, ts], w2b(mi, lo, lo + wid),
                                start=(mi == 0), stop=(mi == NMM - 1))
                        else:
                            s0, p = DR_SEQ[mi - NBF]
                            nc.tensor.matmul(
                                ps, hq[:, s0:s0 + 2, ts],
                                W2qp[p][:, :, lo:lo + wid],
                                start=(mi == 0), stop=(mi == NMM - 1),
                                perf_mode=DR)

                dums = cpool.tile([1, 512], bf16)
                nc.gpsimd.memset(dums[:], 0.0)
                ones = cpool.tile([1, 128], bf16)
                nc.vector.memset(ones[:], 1.0)

                # PE warm-up chain: dummy matmuls gated only on two tiny
                # DVE memsets keep the PE busy-clock running through the
                # initial DMA wait, so the cost model's p-state ramp (~3us
                # of 2x-slow after any idle) is spent on throwaway work and
                # the real stream runs at full rate.  The last two slots
                # broadcast b2 across partitions (ready ~3.8us via the tiny
                # SWDGE b2row DMA), replacing a 1.4us-wide b2bc input DMA
                # on the critical input ladder.
                warm = psum("a1")
                for wd in range(5):
                    nc.tensor.matmul(warm[0:1, 0:512], dums[0:1, 0:1],
                                     dums[0:1, :], start=True, stop=True)
                for vh in range(2):
                    pb = warm[:, vh * 512:(vh + 1) * 512]
                    nc.tensor.matmul(pb, ones[:],
                                     b2row[:, vh * 512:(vh + 1) * 512],
                                     start=True, stop=True)
                    nc.vector.tensor_copy(b2bc[:, vh * 512:(vh + 1) * 512],
                                          pb)

                # hfT chunks 0-2, hgT chunks, then u0 with skewed stages.
                # Layer-1 psums use halves of the two-bank tiles; the pool's
                # 2-buf rotation keeps reuse two allocations apart.
                pf0 = psum("a0")
                hfT_mms(pf0, 0, 0, 0, 5)
                pf1 = psum("a1")
                hfT_mms(pf1, 0, 1, 0, 5)
                hfT_mms(pf0, 0, 0, 5, HC)
                hfT_drain(pf0, 0, 0)
                hfT_mms(pf1, 0, 1, 5, HC)
                hfT_drain(pf1, 0, 1)
                pq = psum("a0")
                hfT_chunk(2, pq, 0)
                nc.vector.tensor_copy(b1sb[:], b1bf)
                hgT_chunk(0, pq, 512)
                ph1 = psum("a1")
                hgT_chunk(1, ph1, 0)
                hgT_chunk(2, ph1, 512)
                pr = psum("a0")
                hgT_chunk(3, pr, 0)
                hgT_chunk(4, pr, 512)
                assert K8 == 4, "u0 early/late act split assumes K8=4 tiling"
                # u0 acts for the early tiles only: DoubleRow pairs (1,2)
                # need just hfT c1/c2, so u0's first three matmul stages can
                # run while W1f c3/c4 and the late W2q pairs still stream in;
                # hfT c3/c4 + the remaining acts slot in mid-u0.
                Hb0, hq0 = act_u(0, slots=(0, 1))

                # u0 skewed stages: tt0 groups run matmul-index mi while
                # tt1 groups run mi-1, giving input DMAs extra lead time.
                G0 = psum("a0")
                sp0 = [(G0[:, 0:512], 0, 512), (G0[:, 512:1024], 512, 512)]
                group_mms(sp0, Hb0, hq0, 0, 0)
                G1 = psum("a1")
                sp1 = [(G1[:, 0:512], 0, 512), (G1[:, 512:1024], 512, 512)]
                for mi in (1, 2):
                    group_mms(sp0, Hb0, hq0, 0, mi)
                    group_mms(sp1, Hb0, hq0, 1, mi - 1)
                pX = psum("a0")
                hfT_chunk(3, pX, 0)
                hfT_chunk(4, pX, 512)
                act_u(0, slots=(2, 3), hb_hq=(Hb0, hq0))
                for mi in (3, 4):
                    group_mms(sp0, Hb0, hq0, 0, mi)
                    group_mms(sp1, Hb0, hq0, 1, mi - 1)
                ot0 = opool.tile([128, V], f16, tag="o0")
                nc.vector.tensor_tensor(ot0[:], G0[:], b2bc[:], add)
                nc.sync.dma_start(out=out_d[0:128, 0, :], in_=ot0[:])
                group_mms(sp1, Hb0, hq0, 1, NMM - 1)
                ot1 = opool.tile([128, V], f16, tag="o1")
                nc.vector.tensor_tensor(ot1[:], G1[:], b2bc[:], add)
                nc.gpsimd.dma_start(out=out_d[128:256, 0, :], in_=ot1[:])

                # ---------------- u = 1 .. U-2 ----------------
                TAG2 = {0: "a0", 1: "a1"}
                for u in range(1, U - 1):
                    Hb, hq = act_u(u)
                    for tt in range(2):
                        big = psum(TAG2[tt])
                        sp = [(big[:, 0:512], 0, 512),
                              (big[:, 512:1024], 512, 512)]
                        for mi in range(NMM):
                            group_mms(sp, Hb, hq, tt, mi)
                        ot = opool.tile([128, V], f16, tag=f"o{tt}")
                        nc.vector.tensor_tensor(ot[:], big[:], b2bc[:], add)
                        # tt1 stores go out via Pool/SWDGE: its descriptor
                        # generation runs on the idle Pool engine instead of
                        # the shared HWDGE, halving both the SP queue load
                        # (~1.3us/DMA) and the HWDGE ladder.
                        eng = nc.sync if tt == 0 else nc.gpsimd
                        eng.dma_start(
                            out=out_d[tt * 128:(tt + 1) * 128, u, :],
                            in_=ot[:])

                # ---------------- final u: short tail ----------------
                # tt=0 keeps the steady shape.  tt=1 splits: bank A is one
                # [128,512] group; bank B runs two sequential [128,256]
                # quarter-groups (one active group per 2KB zero region),
                # so the very last drain+store chain is short.  The last
                # store uses the otherwise-idle Activation HWDGE queue.
                # tt0 keeps the steady shape; tt1 splits its last 512
                # columns across two additional psum BANKS (one from a
                # second a0 allocation) so the three groups interleave
                # without zero-region conflicts and the very last
                # drain+store chain is a short [128,256] piece on the
                # otherwise-idle Activation HWDGE queue.
                u = U - 1
                Hb, hq = act_u(u)
                big = psum(TAG2[0])
                sp = [(big[:, 0:512], 0, 512), (big[:, 512:1024], 512, 512)]
                for mi in range(NMM):
                    group_mms(sp, Hb, hq, 0, mi)
                ot = opool.tile([128, V], f16, tag="o0")
                nc.vector.tensor_tensor(ot[:], big[:], b2bc[:], add)
                nc.sync.dma_start(out=out_d[0:128, u, :], in_=ot[:])

                big1 = psum(TAG2[1])
                bigX = psum(TAG2[0])
                spA = [(big1[:, 0:512], 0, 512),
                       (big1[:, 512:768], 512, 256),
                       (bigX[:, 0:256], 768, 256)]
                for mi in range(NMM):
                    group_mms(spA, Hb, hq, 1, mi)
                # the small bigX piece drains FIRST (DVE is the serializer
                # here) and stores via the idle Activation queue; the other
                # two pieces overlap on sync and Pool/SWDGE.
                ot1 = opool.tile([128, V], f16, tag="o1")
                nc.vector.tensor_tensor(ot1[:, 768:1024], bigX[:, 0:256],
                                        b2bc[:, 768:1024], add)
                nc.scalar.dma_start(out=out_d[128:256, u, 768:1024],
                                    in_=ot1[:, 768:1024])
                nc.vector.tensor_tensor(ot1[:, 0:512], big1[:, 0:512],
                                        b2bc[:, 0:512], add)
                nc.sync.dma_start(out=out_d[128:256, u, 0:512],
                                  in_=ot1[:, 0:512])
                nc.vector.tensor_tensor(ot1[:, 512:768], big1[:, 512:768],
                                        b2bc[:, 512:768], add)
                nc.gpsimd.dma_start(out=out_d[128:256, u, 512:768],
                                    in_=ot1[:, 512:768])
    nc.compile()
    return nc


def _get_nc():
    if "nc" not in _CACHE:
        _CACHE["nc"] = _build_nc()
    return _CACHE["nc"]


def _pack_shared(W1, b1, W2, b2):
    """Partition-major packed weights, shared across cores."""
    import ml_dtypes

    bf16 = ml_dtypes.bfloat16
    e4 = ml_dtypes.float8_e4m3fn
    W1 = np.asarray(W1, dtype=np.float32)
    W2 = np.asarray(W2, dtype=np.float32)
    # W1f: [p, c, h, k] with source index [h*128+p, c*128+k]
    W1fp = np.ascontiguousarray(
        W1[:EH].reshape(HC, 128, JC, 128).transpose(1, 2, 0, 3)
        .reshape(128, JC * HC * 128)).astype(bf16)
    # W1g: pad rows to 384, then [p, c, pc, k]
    W1g = np.zeros((GC * 128, J), dtype=np.float32)
    W1g[:PH] = W1[EH:]
    W1gp = np.ascontiguousarray(
        W1g.reshape(GC, 128, JC, 128).transpose(1, 2, 0, 3)
        .reshape(128, JC * GC * 128)).astype(bf16)
    # bf16 W2 k-tiles: [p, c, v] with source [c*128+p, v]
    W2t = W2.reshape(JC, 128, V).transpose(1, 0, 2)  # [p, c, v]
    W2bf = np.ascontiguousarray(W2t[:, :NBF].reshape(128, NBF * V)).astype(bf16)
    # fp8 tiles: hi/lo split at scale x4 (exact power of two; the act
    # pre-scales H by 1/4 so no drain-side compensation is needed)
    ws = W2t[:, NBF:].astype(bf16).astype(np.float32) * 4.0  # [p, K8, V]
    w_hi = ws.astype(e4)
    w_lo = (ws - w_hi.astype(np.float32)).astype(e4)
    if K8 == 4:
        pairs = [
            np.stack([w_hi[:, 0], w_hi[:, 1]], axis=1),
            np.stack([w_lo[:, 0], w_lo[:, 1]], axis=1),
            np.stack([w_hi[:, 2], w_hi[:, 3]], axis=1),
            np.stack([w_lo[:, 2], w_lo[:, 3]], axis=1),
        ]
    else:
        pairs = [
            np.stack([w_hi[:, 0], w_hi[:, 1]], axis=1),
            np.stack([w_lo[:, 0], w_lo[:, 1]], axis=1),
            np.stack([w_hi[:, 2], w_lo[:, 2]], axis=1),
        ]
    W2qp = np.ascontiguousarray(
        np.stack(pairs, axis=1).reshape(128, len(pairs) * 2 * V))
    b1p = np.asarray(b1, dtype=np.float32).reshape(JC, 128).T.astype(bf16)
    b2p = np.ascontiguousarray(
        np.asarray(b2, dtype=np.float32).reshape(1, V)).astype(bf16)
    return W1fp, W1gp, W2bf, W2qp, b1p, b2p


def _pack_core(f_b, g_b, b1p, W1gp):
    import ml_dtypes

    bf16 = ml_dtypes.bfloat16
    # fT packed: [p, h, t] with source f[t, h*128+p]
    fTp = np.ascontiguousarray(
        f_b.T.reshape(HC, 128, T).transpose(1, 0, 2).reshape(128, HC * T)
    ).astype(bf16)
    # gT packed: pad rows of g^T [PH, U] to 384 = GC*128
    gTfull = np.zeros((GC * 128, U), dtype=np.float32)
    gTfull[:PH] = g_b.T
    gTp = (gTfull.reshape(GC, 128, U).transpose(1, 0, 2)
           .reshape(128, GC * U).astype(bf16))
    # one merged g-side array: [gT | b1 | W1g]
    gside = np.ascontiguousarray(np.concatenate([gTp, b1p, W1gp], axis=1))
    return fTp, gside


def run(f, g, W1, b1, W2, b2, trace=False):
    """Returns (full_output, BassKernelResults)."""
    from concourse.bass_utils import run_bass_kernel_spmd

    nc = _get_nc()

    W1fp, W1gp, W2bf, W2qp, b1p, b2p = _pack_shared(W1, b1, W2, b2)
    f = np.asarray(f, dtype=np.float32)
    g = np.asarray(g, dtype=np.float32)

    in_maps = []
    for i in range(N_CORES):
        fTp, gside = _pack_core(f[i], g[i], b1p, W1gp)
        in_maps.append({
            "fTp": fTp,
            "gside": gside,
            "W1fp": W1fp,
            "W2bf": W2bf,
            "W2q": W2qp,
            "b2p": b2p,
        })
    res = run_bass_kernel_spmd(nc, in_maps, list(range(N_CORES)), trace=trace)
    out = np.stack([np.asarray(res.results[i]["out"], dtype=np.float32)
                    for i in range(N_CORES)], axis=0)
    return out, res


def kernel(f, g, W1, b1, W2, b2):
    out, _ = run(f, g, W1, b1, W2, b2)
    return out
